# revision 2
# baseline (speedup 1.0000x reference)
"""Deformable attention kernel for 8 Trainium2 NeuronCores (SPMD, batch+head parallel).

Sharding: 16 (batch, head) pairs -> 2 per core. Core c handles batch c//4,
heads 2*(c%4), 2*(c%4)+1. No collectives: each core produces a partial
output projection (over its 128 head-channels); host sums the 4 partials
per batch and adds the bias terms.

Math reformulation of the deformable point-weight + window mask (exact):
  With start = anchor - duration, end = anchor + duration,
  L' = min(r - (start-1), 1), R' = min((end+1) - r, 1), tent = relu(1-|r-anchor|):
  T = relu(tent) ... T = relu(1-|r-anchor|) + L'*R' equals pointweight * window
  indicator wherever positive; numerator N = exp(S*relu(T)) * [T>0]; rows with
  all-masked windows (Z=0) fall back to uniform 1/T attention, matching
  softmax of an all -1e8 row in the reference.
"""
import numpy as np

B, T, E, NH = 2, 1024, 512, 8
HD = E // NH          # 64
N_CORES = 8
HPC = 2               # heads per core

_nc_cache = {}


def _build_program():
    import concourse.bacc as bacc
    import concourse.mybir as mybir
    import concourse.tile as tile
    from concourse.masks import make_identity
    from concourse.bass import ts as bts

    f32 = mybir.dt.float32
    fp16 = mybir.dt.float16
    i32 = mybir.dt.int32
    Alu = mybir.AluOpType
    Act = mybir.ActivationFunctionType

    nc = bacc.Bacc(None, target_bir_lowering=False)

    xT16 = nc.declare_dram_parameter("xT16", [E, T], fp16, isOutput=False)
    xT32 = nc.declare_dram_parameter("xT32", [E, T], f32, isOutput=False)
    wq = nc.declare_dram_parameter("wq", [E, 128], fp16, isOutput=False)
    wk = nc.declare_dram_parameter("wk", [E, 128], fp16, isOutput=False)
    wv = nc.declare_dram_parameter("wv", [E, 128], fp16, isOutput=False)
    wc = nc.declare_dram_parameter("wc", [E, 4], f32, isOutput=False)
    bc = nc.declare_dram_parameter("bc", [1, 4], f32, isOutput=False)
    bq8 = nc.declare_dram_parameter("bq8", [128, 1], f32, isOutput=False)
    bkc = nc.declare_dram_parameter("bkc", [128, 1], f32, isOutput=False)
    wout = nc.declare_dram_parameter("wout", [128, E], fp16, isOutput=False)
    y = nc.declare_dram_parameter("y", [T, E], fp16, isOutput=True)

    with tile.TileContext(nc) as tc:
        with tc.tile_pool(name="const", bufs=1) as const, \
             tc.tile_pool(name="big", bufs=1) as big, \
             tc.tile_pool(name="cols", bufs=1) as cols:

            # ---------- constants ----------
            it_i = const.tile([128, T], i32)
            nc.gpsimd.iota(it_i, pattern=[[1, T]], base=0, channel_multiplier=0)
            I16 = const.tile([128, T], fp16)
            nc.vector.tensor_copy(I16, it_i)
            qx_i = const.tile([128, 8], i32)
            nc.gpsimd.iota(qx_i, pattern=[[128, 8]], base=0, channel_multiplier=1)
            qidx = const.tile([128, 8], f32)
            nc.vector.tensor_copy(qidx, qx_i)
            ident = const.tile([128, 128], fp16)
            make_identity(nc, ident)
            ones1 = const.tile([1, 128], f32)
            nc.vector.memset(ones1, 1.0)

            # ---------- input loads ----------
            xt16 = big.tile([128, 4, T], fp16)
            nc.sync.dma_start(xt16, xT16.ap().rearrange("(j p) t -> p j t", p=128))
            xt32 = big.tile([128, 4, T], f32)
            nc.sync.dma_start(xt32, xT32.ap().rearrange("(j p) t -> p j t", p=128))
            wq_sb = big.tile([128, 4, 128], fp16)
            nc.sync.dma_start(wq_sb, wq.ap().rearrange("(j p) m -> p j m", p=128))
            wk_sb = big.tile([128, 4, 128], fp16)
            nc.sync.dma_start(wk_sb, wk.ap().rearrange("(j p) m -> p j m", p=128))
            wv_sb = big.tile([128, 4, 128], fp16)
            nc.sync.dma_start(wv_sb, wv.ap().rearrange("(j p) m -> p j m", p=128))
            wc_sb = big.tile([128, 4, 4], f32)
            nc.sync.dma_start(wc_sb, wc.ap().rearrange("(j p) m -> p j m", p=128))
            bc_sb = big.tile([1, 4], f32)
            nc.sync.dma_start(bc_sb, bc.ap())
            bq8_sb = cols.tile([128, 1], f32)
            nc.sync.dma_start(bq8_sb, bq8.ap())
            bk_sb = cols.tile([128, 1], f32)
            nc.sync.dma_start(bk_sb, bkc.ap())
            wout_sb = big.tile([128, E], fp16)
            nc.sync.dma_start(wout_sb, wout.ap())

            # ---------- setup phase: od + Q^T/K^T/V projections ----------
            with tc.tile_pool(name="ps_setup", bufs=1, space="PSUM") as pss:
                od_ps = pss.tile([128, 8, 4], f32)
                for j2 in range(8):
                    for jc in range(4):
                        nc.tensor.matmul(od_ps[:, j2, :],
                                         xt32[:, jc, bts(j2, 128)],
                                         wc_sb[:, jc, :],
                                         start=(jc == 0), stop=False)
                    nc.tensor.matmul(od_ps[:, j2, :], ones1, bc_sb,
                                     start=False, stop=True)

                # offsets/durations -> per-(tile, head) scalar columns, f32
                th = cols.tile([128, 8, 2], f32)
                nc.scalar.activation(th, od_ps[:, :, 0:2], Act.Tanh)
                du2 = cols.tile([128, 8, 2], f32)
                nc.scalar.activation(du2, od_ps[:, :, 2:4], Act.Tanh, scale=0.5)
                an = cols.tile([128, 8, 2], f32)
                for h2 in range(2):
                    nc.vector.scalar_tensor_tensor(an[:, :, h2], th[:, :, h2],
                                                   1024.0, qidx,
                                                   op0=Alu.mult, op1=Alu.add)
                durp1 = cols.tile([128, 8, 2], f32)
                nc.vector.tensor_scalar(durp1, du2, 512.0, 513.0,
                                        op0=Alu.mult, op1=Alu.add)
                sm1n = cols.tile([128, 8, 2], f32)
                nc.vector.tensor_tensor(sm1n, durp1, an, op=Alu.subtract)
                ep1 = cols.tile([128, 8, 2], f32)
                nc.vector.tensor_tensor(ep1, durp1, an, op=Alu.add)
                anp1 = cols.tile([128, 8, 2], f32)
                nc.vector.tensor_scalar(anp1, an, 1.0, None, op0=Alu.add)
                anm1 = cols.tile([128, 8, 2], f32)
                nc.vector.tensor_scalar(anm1, an, 1.0, None, op0=Alu.subtract)

                # Q^T (both heads, scaled by 1/8 with bias) and K^T
                qt_ps = pss.tile([128, T], f32)
                for n2 in range(2):
                    for jc in range(4):
                        nc.tensor.matmul(qt_ps[:, bts(n2, 512)],
                                         wq_sb[:, jc, :],
                                         xt16[:, jc, bts(n2, 512)],
                                         start=(jc == 0), stop=(jc == 3))
                qt16 = big.tile([128, T], fp16)
                nc.scalar.activation(qt16, qt_ps, Act.Identity,
                                     bias=bq8_sb, scale=0.125)
                kt_ps = pss.tile([128, T], f32)
                for n2 in range(2):
                    for jc in range(4):
                        nc.tensor.matmul(kt_ps[:, bts(n2, 512)],
                                         wk_sb[:, jc, :],
                                         xt16[:, jc, bts(n2, 512)],
                                         start=(jc == 0), stop=(jc == 3))
                kt16 = big.tile([128, T], fp16)
                nc.scalar.activation(kt16, kt_ps, Act.Identity,
                                     bias=bk_sb, scale=1.0)
                v_ps = pss.tile([128, 8, 128], f32)
                for j2 in range(8):
                    for jc in range(4):
                        nc.tensor.matmul(v_ps[:, j2, :],
                                         xt16[:, jc, bts(j2, 128)],
                                         wv_sb[:, jc, :],
                                         start=(jc == 0), stop=(jc == 3))
                v16 = big.tile([128, 8, 128], fp16)
                nc.scalar.activation(v16, v_ps, Act.Copy)

            # ---------- main loop ----------
            with tc.tile_pool(name="ps_s", bufs=2, space="PSUM") as ps_s, \
                 tc.tile_pool(name="ps_pt", bufs=1, space="PSUM") as ps_pt, \
                 tc.tile_pool(name="ps_at", bufs=1, space="PSUM") as ps_at, \
                 tc.tile_pool(name="ps_y", bufs=1, space="PSUM") as ps_y, \
                 tc.tile_pool(name="work", bufs=2) as work, \
                 tc.tile_pool(name="out", bufs=2) as outp:
                for it in range(8):
                    at_ps = ps_at.tile([128, 128], f32)
                    for h2 in range(2):
                        hs = slice(64 * h2, 64 * (h2 + 1))
                        s_ps = ps_s.tile([128, T], f32)
                        for n2 in range(2):
                            nc.tensor.matmul(s_ps[:, bts(n2, 512)],
                                             qt16[hs, bts(it, 128)],
                                             kt16[hs, bts(n2, 512)],
                                             start=True, stop=True)
                        c_sm1n = sm1n[:, it, h2:h2 + 1]
                        c_ep1 = ep1[:, it, h2:h2 + 1]
                        c_anp1 = anp1[:, it, h2:h2 + 1]
                        c_anm1 = anm1[:, it, h2:h2 + 1]

                        Lp = work.tile([128, T], fp16)
                        nc.vector.tensor_scalar(Lp, I16, c_sm1n, 1.0,
                                                op0=Alu.add, op1=Alu.min)
                        Rn = work.tile([128, T], fp16)
                        nc.vector.tensor_scalar(Rn, I16, c_ep1, -1.0,
                                                op0=Alu.subtract, op1=Alu.max)
                        q1 = work.tile([128, T], fp16)
                        nc.vector.tensor_scalar(q1, I16, c_anp1, -1.0,
                                                op0=Alu.subtract, op1=Alu.mult)
                        q2 = work.tile([128, T], fp16)
                        nc.vector.tensor_scalar(q2, I16, c_anm1, None,
                                                op0=Alu.subtract)
                        LRn = work.tile([128, T], fp16)
                        nc.vector.tensor_tensor(LRn, Lp, Rn, op=Alu.mult)
                        tentU = work.tile([128, T], fp16)
                        nc.vector.tensor_tensor(tentU, q1, q2, op=Alu.min)
                        Tm = work.tile([128, T], fp16)
                        nc.vector.scalar_tensor_tensor(Tm, tentU, 0.0, LRn,
                                                       op0=Alu.max,
                                                       op1=Alu.subtract)
                        V1 = work.tile([128, T], fp16)
                        nc.vector.scalar_tensor_tensor(V1, Tm, 0.0, s_ps,
                                                       op0=Alu.max,
                                                       op1=Alu.mult)
                        E0 = work.tile([128, T], fp16)
                        nc.scalar.activation(E0, V1, Act.Exp)
                        Nt = work.tile([128, T], fp16)
                        Zc = work.tile([128, 1], f32)
                        nc.vector.scalar_tensor_tensor(Nt, Tm, 0.0, E0,
                                                       op0=Alu.is_gt,
                                                       op1=Alu.mult,
                                                       accum_out=Zc)
                        U = work.tile([128, 1], f32)
                        nc.vector.tensor_scalar(U, Zc, 0.0, None,
                                                op0=Alu.is_equal)
                        Z2 = work.tile([128, 1], f32)
                        nc.vector.scalar_tensor_tensor(Z2, U, 1024.0, Zc,
                                                       op0=Alu.mult,
                                                       op1=Alu.add)
                        Zi = work.tile([128, 1], f32)
                        nc.vector.reciprocal(Zi, Z2)
                        Pw = work.tile([128, T], fp16)
                        nc.vector.tensor_scalar(Pw, Nt, U, Zi,
                                                op0=Alu.add, op1=Alu.mult)

                        pt_ps = ps_pt.tile([128, 8, 128], fp16)
                        for j in range(8):
                            nc.tensor.transpose(pt_ps[:, j, :],
                                                Pw[:, bts(j, 128)], ident)
                        pt16 = work.tile([128, 8, 128], fp16)
                        nc.scalar.activation(pt16, pt_ps, Act.Copy)
                        for j in range(8):
                            nc.tensor.matmul(at_ps[hs, :],
                                             v16[:, j, hs],
                                             pt16[:, j, :],
                                             start=(j == 0), stop=(j == 7))
                    at16 = outp.tile([128, 128], fp16)
                    nc.scalar.activation(at16, at_ps, Act.Copy)
                    y_ps = ps_y.tile([128, E], f32)
                    nc.tensor.matmul(y_ps, at16, wout_sb, start=True, stop=True)
                    y16 = outp.tile([128, E], fp16)
                    nc.vector.tensor_copy(y16, y_ps)
                    nc.sync.dma_start(y.ap()[bts(it, 128), :], y16)

    nc.finalize()
    return nc


def _prep_in_maps(x, W_qkv, b_qkv, W_od, b_od, W_out, b_out):
    x = np.asarray(x, np.float32)
    W_qkv = np.asarray(W_qkv, np.float32)
    b_qkv = np.asarray(b_qkv, np.float32)
    W_od = np.asarray(W_od, np.float32)
    b_od = np.asarray(b_od, np.float32)

    Wc_full = W_qkv[:, :E] @ W_od                    # (512, 16)
    bc_full = b_qkv[:E] @ W_od + b_od                # (16,)

    in_maps = []
    for core in range(N_CORES):
        b = core // 4
        h0 = HPC * (core % 4)
        qs = slice(h0 * HD, (h0 + HPC) * HD)         # 128 cols
        xt = np.ascontiguousarray(x[b].T)            # (512, 1024)
        odc = [h0, h0 + 1, NH + h0, NH + h0 + 1]
        in_maps.append({
            "xT16": xt.astype(np.float16),
            "xT32": xt,
            "wq": np.ascontiguousarray(W_qkv[:, qs]).astype(np.float16),
            "wk": np.ascontiguousarray(W_qkv[:, E:][:, qs]).astype(np.float16),
            "wv": np.ascontiguousarray(W_qkv[:, 2 * E:][:, qs]).astype(np.float16),
            "wc": np.ascontiguousarray(Wc_full[:, odc]),
            "bc": np.ascontiguousarray(bc_full[odc])[None, :],
            "bq8": (b_qkv[:E][qs] / 8.0).reshape(128, 1).astype(np.float32),
            "bkc": b_qkv[E:2 * E][qs].reshape(128, 1).astype(np.float32),
            "wout": np.ascontiguousarray(W_out[qs, :]).astype(np.float16),
        })
    return in_maps


def kernel(x, W_qkv, b_qkv, W_od, b_od, W_out, b_out, length):
    from concourse.bass_utils import run_bass_kernel_spmd

    assert int(length) == T
    if "nc" not in _nc_cache:
        _nc_cache["nc"] = _build_program()
    nc = _nc_cache["nc"]

    in_maps = _prep_in_maps(x, W_qkv, b_qkv, W_od, b_od, W_out, b_out)
    res = run_bass_kernel_spmd(nc, in_maps, list(range(N_CORES)))

    W_out = np.asarray(W_out, np.float32)
    b_out = np.asarray(b_out, np.float32)
    b_qkv = np.asarray(b_qkv, np.float32)
    out = np.zeros((B, T, E), np.float32)
    for core in range(N_CORES):
        out[core // 4] += res.results[core]["y"].astype(np.float32)
    out += b_qkv[2 * E:] @ W_out + b_out
    return out
